# revision 6
# baseline (speedup 1.0000x reference)
"""ASPP pooling head on Trainium2 (Bass/Tile), data-parallel over batch on 8 cores.

Computation per sample:
    pooled = mean(x, spatial)            # [Cin]
    y      = relu((pooled @ W.T) * bn_scale + bn_shift)   # [Cout]
    out    = broadcast(y, spatial)       # [Cout, H, W]

Device kernel per core (2 samples), engine assignment:
    - SyncE   : x input stream, [128ch, 2, 4096] f32 tiles (4 MiB DMAs)
    - VectorE : spatial-sum of segment 0 of each tile (tensor_reduce),
                broadcast of o-block 1 (tensor_scalar add+max)
    - ScalarE : spatial-sum of segment 1 (activation Identity + accum_out),
                broadcast of o-block 0 (activation Relu, step-0 src AP)
    - TensorE : 16 accumulating matmuls per (sample, o-block):
                psum[128o, 1] += wt_chunk[128c, 128o].T @ pooled[128c, 1],
                wt = (W * bn_scale / 4096).T folded on host
    - GpSimd  : const loads + output stores (SWDGE, own queue)
"""

import numpy as np

B, CIN, H, W_SP = 16, 2048, 64, 64
COUT = 256
NCORES = 8
BPC = B // NCORES          # samples per core
SP = H * W_SP              # 4096 spatial positions
KCH = CIN // 128           # 16 channel chunks of 128
CPT = 2                    # channel chunks per x tile (4 MiB DMA)
NOB = COUT // 128          # output-channel blocks
HSP = SP // 2              # broadcast/store half width
BN_EPS = 1e-5

_CACHE = {}


def _build_nc():
    import concourse.bacc as bacc
    import concourse.mybir as mybir
    import concourse.tile as tile

    nc = bacc.Bacc("TRN2", target_bir_lowering=False, debug=False,
                   num_devices=NCORES)
    f32 = mybir.dt.float32
    AT = mybir.ActivationFunctionType
    x = nc.dram_tensor("x", [BPC, CIN, SP], f32, kind="ExternalInput").ap()
    wt = nc.dram_tensor("wt", [CIN, COUT], f32, kind="ExternalInput").ap()
    shift = nc.dram_tensor("shift", [COUT], f32, kind="ExternalInput").ap()
    out = nc.dram_tensor("out", [BPC, COUT, SP], f32, kind="ExternalOutput").ap()

    with tile.TileContext(nc) as tc, \
         tc.tile_pool(name="consts", bufs=1) as consts, \
         tc.tile_pool(name="xin", bufs=3) as xin, \
         tc.tile_pool(name="pooled", bufs=8) as pooledp, \
         tc.tile_pool(name="psum", bufs=2, space="PSUM") as psump, \
         tc.tile_pool(name="bcast", bufs=3) as bcastp:

        # Constants via GpSimd/SWDGE so the Sync input queue starts on x
        # immediately. wt laid out [128 (c within chunk), KCH, COUT]; chunk
        # k's o-block ob is columns k*COUT + ob*128 ...
        wt_sb = consts.tile([128, KCH * COUT], f32)
        nc.gpsimd.dma_start(wt_sb[:].rearrange("p (k o) -> p k o", k=KCH),
                            wt.rearrange("(k p) o -> p k o", p=128))
        shift_sb = consts.tile([128, NOB], f32)
        nc.gpsimd.dma_start(shift_sb[:], shift.rearrange("(ob p) -> p ob", p=128))
        zeros_col = consts.tile([128, 1], f32)
        nc.gpsimd.memset(zeros_col[:], 0.0)
        scratch = consts.tile([128, SP], f32)  # ACT accum dummy elementwise out

        for b in range(BPC):
            pss = [psump.tile([128, 1], f32, name=f"ps{ob}", tag=f"ps{ob}")
                   for ob in range(NOB)]
            for kt in range(KCH // CPT):
                xt = xin.tile([128, CPT, SP], f32)
                src = x[b, kt * CPT * 128:(kt + 1) * CPT * 128, :] \
                    .rearrange("(c p) s -> p c s", p=128)
                nc.sync.dma_start(xt[:], src)
                # segment 0 -> VectorE reduce; segment 1 -> ScalarE accum
                ptv = pooledp.tile([128, 1], f32, name="ptv", tag="ptv")
                nc.vector.reduce_sum(ptv[:], xt[:, 0, :],
                                     axis=mybir.AxisListType.X)
                pta = pooledp.tile([128, 1], f32, name="pta", tag="pta")
                nc.scalar.activation(scratch[:], xt[:, 1, :], AT.Identity,
                                     bias=zeros_col[:], scale=1.0,
                                     accum_out=pta[:])
                for c, pt in ((0, ptv), (1, pta)):
                    k = kt * CPT + c
                    for ob in range(NOB):
                        nc.tensor.matmul(
                            pss[ob][:],
                            lhsT=wt_sb[:, k * COUT + ob * 128:
                                       k * COUT + ob * 128 + 128],
                            rhs=pt[:, 0:1],
                            start=(k == 0),
                            stop=(k == KCH - 1),
                        )
            # Broadcast + store; ob0 on ScalarE, ob1 on VectorE, in halves
            # so stores (GpSimd SWDGE) begin before the full row is done.
            for ob in range(NOB):
                for h in range(2):
                    bc = bcastp.tile([128, HSP], f32, name=f"bc{ob}",
                                     tag=f"bc{ob}")
                    src_b = pss[ob][:].broadcast_to([128, HSP])
                    if ob == 0:
                        nc.scalar.activation(bc[:], src_b, AT.Relu,
                                             bias=shift_sb[:, 0:1], scale=1.0)
                    else:
                        nc.vector.tensor_scalar(
                            out=bc[:], in0=src_b,
                            scalar1=shift_sb[:, ob:ob + 1], scalar2=0.0,
                            op0=mybir.AluOpType.add, op1=mybir.AluOpType.max)
                    nc.gpsimd.dma_start(
                        out[b, ob * 128:(ob + 1) * 128,
                            h * HSP:(h + 1) * HSP], bc[:])

    nc.compile()
    return nc


def _prep_inputs(x, W, gamma, beta, running_mean, running_var):
    scale = np.asarray(gamma, np.float32) / np.sqrt(
        np.asarray(running_var, np.float32) + np.float32(BN_EPS))
    wt = np.ascontiguousarray(
        (np.asarray(W, np.float32) * scale[:, None]).T / np.float32(SP))
    shift = (np.asarray(beta, np.float32)
             - np.asarray(running_mean, np.float32) * scale).astype(np.float32)
    xs = np.ascontiguousarray(np.asarray(x, np.float32)).reshape(
        NCORES, BPC, CIN, SP)
    return [{"x": xs[i], "wt": wt, "shift": shift} for i in range(NCORES)]


def kernel(x, W, gamma, beta, running_mean, running_var):
    from concourse import bass_utils

    if "nc" not in _CACHE:
        _CACHE["nc"] = _build_nc()
    nc = _CACHE["nc"]
    in_maps = _prep_inputs(x, W, gamma, beta, running_mean, running_var)
    res = bass_utils.run_bass_kernel_spmd(nc, in_maps,
                                          core_ids=list(range(NCORES)))
    outs = [res.results[i]["out"] for i in range(NCORES)]
    return np.concatenate(outs, axis=0).reshape(B, COUT, H, W_SP)


# revision 9
# speedup vs baseline: 1.0302x; 1.0302x over previous
"""ASPP pooling head on Trainium2 (Bass/Tile), data-parallel over batch on 8 cores.

Computation per sample:
    pooled = mean(x, spatial)            # [Cin]
    y      = relu((pooled @ W.T) * bn_scale + bn_shift)   # [Cout]
    out    = broadcast(y, spatial)       # [Cout, H, W]

Device kernel per core (2 samples), engine assignment:
    - SyncE   : x input stream, [128ch, 2, 4096] f32 tiles (4 MiB DMAs)
    - VectorE : spatial-sum of segment 0 of each tile (tensor_reduce),
                broadcast of o-block 1 (tensor_scalar add+max)
    - ScalarE : spatial-sum of segment 1 (activation Identity + accum_out),
                broadcast of o-block 0 (activation Relu, step-0 src AP)
    - TensorE : 16 accumulating matmuls per (sample, o-block):
                psum[128o, 1] += wt_chunk[128c, 128o].T @ pooled[128c, 1],
                wt = (W * bn_scale / 4096).T folded on host
    - GpSimd  : const loads + output stores (SWDGE, own queue)
"""

import numpy as np

B, CIN, H, W_SP = 16, 2048, 64, 64
COUT = 256
NCORES = 8
BPC = B // NCORES          # samples per core
SP = H * W_SP              # 4096 spatial positions
KCH = CIN // 128           # 16 channel chunks of 128
CPT = 2                    # channel chunks per x tile (4 MiB DMA)
NOB = COUT // 128          # output-channel blocks
HSP = SP // 2              # broadcast/store half width
BN_EPS = 1e-5

_CACHE = {}


def _build_nc():
    import concourse.bacc as bacc
    import concourse.mybir as mybir
    import concourse.tile as tile

    nc = bacc.Bacc("TRN2", target_bir_lowering=False, debug=False,
                   num_devices=NCORES)
    f32 = mybir.dt.float32
    AT = mybir.ActivationFunctionType
    x = nc.dram_tensor("x", [BPC, CIN, SP], f32, kind="ExternalInput").ap()
    wt = nc.dram_tensor("wt", [CIN, COUT], f32, kind="ExternalInput").ap()
    shift = nc.dram_tensor("shift", [COUT], f32, kind="ExternalInput").ap()
    out = nc.dram_tensor("out", [BPC, COUT, SP], f32, kind="ExternalOutput").ap()

    with tile.TileContext(nc) as tc, \
         tc.tile_pool(name="consts", bufs=1) as consts, \
         tc.tile_pool(name="xin", bufs=4) as xin, \
         tc.tile_pool(name="pooled", bufs=8) as pooledp, \
         tc.tile_pool(name="psum", bufs=2, space="PSUM") as psump, \
         tc.tile_pool(name="bcast", bufs=2) as bcastp:

        # Constants via GpSimd/SWDGE so the Sync input queue starts on x
        # immediately. wt laid out [128 (c within chunk), KCH, COUT]; chunk
        # k's o-block ob is columns k*COUT + ob*128 ...
        wt_sb = consts.tile([128, KCH * COUT], f32)
        nc.gpsimd.dma_start(wt_sb[:].rearrange("p (k o) -> p k o", k=KCH),
                            wt.rearrange("(k p) o -> p k o", p=128))
        shift_sb = consts.tile([128, NOB], f32)
        nc.gpsimd.dma_start(shift_sb[:], shift.rearrange("(ob p) -> p ob", p=128))
        zeros_col = consts.tile([128, 1], f32)
        nc.gpsimd.memset(zeros_col[:], 0.0)
        scratch = consts.tile([128, SP], f32)  # ACT accum dummy elementwise out

        for b in range(BPC):
            pss = [psump.tile([128, 1], f32, name=f"ps{ob}", tag=f"ps{ob}")
                   for ob in range(NOB)]
            for kt in range(KCH // CPT):
                xt = xin.tile([128, CPT, SP], f32)
                src = x[b, kt * CPT * 128:(kt + 1) * CPT * 128, :] \
                    .rearrange("(c p) s -> p c s", p=128)
                nc.sync.dma_start(xt[:], src)
                # segment 0 -> VectorE reduce; segment 1 -> ScalarE accum
                ptv = pooledp.tile([128, 1], f32, name="ptv", tag="ptv")
                nc.vector.reduce_sum(ptv[:], xt[:, 0, :],
                                     axis=mybir.AxisListType.X)
                pta = pooledp.tile([128, 1], f32, name="pta", tag="pta")
                nc.scalar.activation(scratch[:], xt[:, 1, :], AT.Identity,
                                     bias=zeros_col[:], scale=1.0,
                                     accum_out=pta[:])
                for c, pt in ((0, ptv), (1, pta)):
                    k = kt * CPT + c
                    for ob in range(NOB):
                        nc.tensor.matmul(
                            pss[ob][:],
                            lhsT=wt_sb[:, k * COUT + ob * 128:
                                       k * COUT + ob * 128 + 128],
                            rhs=pt[:, 0:1],
                            start=(k == 0),
                            stop=(k == KCH - 1),
                        )
            # Broadcast + store; ob0 on ScalarE, ob1 on VectorE, in halves
            # so stores (GpSimd SWDGE) begin before the full row is done.
            for ob in range(NOB):
                for h in range(2):
                    bc = bcastp.tile([128, HSP], f32, name=f"bc{ob}",
                                     tag=f"bc{ob}")
                    src_b = pss[ob][:].broadcast_to([128, HSP])
                    if ob == 0:
                        nc.scalar.activation(bc[:], src_b, AT.Relu,
                                             bias=shift_sb[:, 0:1], scale=1.0)
                    else:
                        nc.vector.tensor_scalar(
                            out=bc[:], in0=src_b,
                            scalar1=shift_sb[:, ob:ob + 1], scalar2=0.0,
                            op0=mybir.AluOpType.add, op1=mybir.AluOpType.max)
                    nc.scalar.dma_start(
                        out[b, ob * 128:(ob + 1) * 128,
                            h * HSP:(h + 1) * HSP], bc[:])

    nc.compile()
    return nc


def _prep_inputs(x, W, gamma, beta, running_mean, running_var):
    scale = np.asarray(gamma, np.float32) / np.sqrt(
        np.asarray(running_var, np.float32) + np.float32(BN_EPS))
    wt = np.ascontiguousarray(
        (np.asarray(W, np.float32) * scale[:, None]).T / np.float32(SP))
    shift = (np.asarray(beta, np.float32)
             - np.asarray(running_mean, np.float32) * scale).astype(np.float32)
    xs = np.ascontiguousarray(np.asarray(x, np.float32)).reshape(
        NCORES, BPC, CIN, SP)
    return [{"x": xs[i], "wt": wt, "shift": shift} for i in range(NCORES)]


def kernel(x, W, gamma, beta, running_mean, running_var):
    from concourse import bass_utils

    if "nc" not in _CACHE:
        _CACHE["nc"] = _build_nc()
    nc = _CACHE["nc"]
    in_maps = _prep_inputs(x, W, gamma, beta, running_mean, running_var)
    res = bass_utils.run_bass_kernel_spmd(nc, in_maps,
                                          core_ids=list(range(NCORES)))
    outs = [res.results[i]["out"] for i in range(NCORES)]
    return np.concatenate(outs, axis=0).reshape(B, COUT, H, W_SP)


# revision 22
# speedup vs baseline: 1.2014x; 1.1661x over previous
"""ASPP pooling head on Trainium2 (Bass/Tile), data-parallel over batch on 8 cores.

Computation per sample:
    pooled = mean(x, spatial)            # [Cin]
    y      = relu((pooled @ W.T) * bn_scale + bn_shift)   # [Cout]
    out    = broadcast(y, spatial)       # [Cout, H, W]

Device kernel per core (2 samples), winning config (measured floor ~206 us,
~= 77.6 MB of mandatory HBM traffic at the ~420 GB/s per-core streaming rate
plus fixed NEFF overhead; f32 end-to-end, rel err ~1e-6):
    - x streamed as 32x [128ch, 4096] f32 tiles (2 MiB DMAs, 9-deep pool)
      on SyncE HWDGE; fine granularity + deep queue rides out HBM-stack
      contention from the paired core
    - spatial sums on VectorE tensor_reduce (free-dim, 1x mode, ~4.4 us/tile)
    - 16 accumulating PE matmuls per (sample, o-block):
      psum[128o, 1] += wt_chunk[128c, 128o].T @ pooled[128c, 1],
      with wt = (W * bn_scale / 4096).T folded on host
    - broadcast = one pass over the output bytes: Relu(psum_bcast + shift)
      via ScalarE activation with a stride-0 source AP; stores via ScalarE
      HWDGE (second ring, no head-of-line blocking with input stream)
"""

import numpy as np

B, CIN, H, W_SP = 16, 2048, 64, 64
COUT = 256
NCORES = 8
BPC = B // NCORES          # samples per core
SP = H * W_SP              # 4096 spatial positions
KCH = CIN // 128           # 16 channel chunks of 128
CPT = 2                    # channel chunks per x tile (4 MiB DMA)
NOB = COUT // 128          # output-channel blocks
BN_EPS = 1e-5

_CACHE = {}

VARIANTS = {
    "v1": dict(split_reduce=False, bcast_halves=1, bcast_dve_ob1=False,
               consts_gpsimd=False, xin_bufs=4),
    "v3": dict(split_reduce=True, bcast_halves=2, bcast_dve_ob1=True,
               consts_gpsimd=True, xin_bufs=4),
    "v4": dict(split_reduce=True, bcast_halves=2, bcast_dve_ob1=True,
               consts_gpsimd=True, xin_bufs=4, alt_in_queue=True),
    "v5": dict(split_reduce=False, bcast_halves=2, bcast_dve_ob1=False,
               consts_gpsimd=True, xin_bufs=4),
    "v5d": dict(split_reduce=False, bcast_halves=2, bcast_dve_ob1=True,
                consts_gpsimd=True, xin_bufs=4),
    "v1g": dict(split_reduce=False, bcast_halves=1, bcast_dve_ob1=False,
                consts_gpsimd=True, xin_bufs=4),
    "v1c1": dict(split_reduce=False, bcast_halves=1, bcast_dve_ob1=False,
                 consts_gpsimd=False, xin_bufs=8, cpt=1),
    "v1c4": dict(split_reduce=False, bcast_halves=1, bcast_dve_ob1=False,
                 consts_gpsimd=False, xin_bufs=2, cpt=4),
    "v1c1b": dict(split_reduce=False, bcast_halves=1, bcast_dve_ob1=False,
                  consts_gpsimd=False, xin_bufs=9, cpt=1),
    "v1h1": dict(split_reduce=False, bcast_halves=1, bcast_dve_ob1=False,
                 consts_gpsimd=False, xin_bufs=8, cpt=1, dma_splits=2),
    "v1c1c": dict(split_reduce=False, bcast_halves=2, bcast_dve_ob1=False,
                  consts_gpsimd=False, xin_bufs=10, cpt=1),
    "v1c1q": dict(split_reduce=False, bcast_halves=2, bcast_dve_ob1=False,
                  consts_gpsimd=False, xin_bufs=10, cpt=1, alt_in_queue=True),
    "v1c1s6": dict(split_reduce=False, bcast_halves=1, bcast_dve_ob1=False,
                   consts_gpsimd=False, xin_bufs=6, cpt=1),
    "v6": dict(split_reduce=False, bcast_halves=1, bcast_dve_ob1=False,
               consts_gpsimd=False, xin_bufs=8, cpt=1, alt_reduce=True),
    "v7": dict(split_reduce=False, bcast_halves=1, bcast_dve_ob1=True,
               consts_gpsimd=False, xin_bufs=8, cpt=1, alt_reduce=True,
               consts_late=True),
}


def _build_nc(split_reduce=False, bcast_halves=1, bcast_dve_ob1=False,
              consts_gpsimd=False, xin_bufs=9, cpt=1,
              alt_in_queue=False, dma_splits=1, alt_reduce=False,
              consts_late=False):
    import concourse.bacc as bacc
    import concourse.mybir as mybir
    import concourse.tile as tile

    nc = bacc.Bacc("TRN2", target_bir_lowering=False, debug=False,
                   num_devices=NCORES)
    f32 = mybir.dt.float32
    AT = mybir.ActivationFunctionType
    x = nc.dram_tensor("x", [BPC, CIN, SP], f32, kind="ExternalInput").ap()
    wt = nc.dram_tensor("wt", [CIN, COUT], f32, kind="ExternalInput").ap()
    shift = nc.dram_tensor("shift", [COUT], f32, kind="ExternalInput").ap()
    out = nc.dram_tensor("out", [BPC, COUT, SP], f32, kind="ExternalOutput").ap()

    hsp = SP // bcast_halves
    cdma = nc.gpsimd.dma_start if consts_gpsimd else nc.sync.dma_start

    with tile.TileContext(nc) as tc, \
         tc.tile_pool(name="consts", bufs=1) as consts, \
         tc.tile_pool(name="xin", bufs=xin_bufs) as xin, \
         tc.tile_pool(name="pooled", bufs=8) as pooledp, \
         tc.tile_pool(name="psum", bufs=2, space="PSUM") as psump, \
         tc.tile_pool(name="bcast", bufs=2) as bcastp:

        # wt laid out [128 (c within chunk), KCH, COUT]; chunk k's o-block ob
        # is columns k*COUT + ob*128 ...  Emission may be deferred into the
        # first tiles' DMA stream (consts_late) to keep the ramp on x.
        wt_sb = consts.tile([128, KCH * COUT], f32)
        shift_sb = consts.tile([128, NOB], f32)
        wt_r = wt.rearrange("(k p) o -> p k o", p=128)
        wt_d = wt_sb[:].rearrange("p (k o) -> p k o", k=KCH)
        hk = KCH // 2

        def emit_consts(step):
            if not consts_late and step == 0:
                cdma(wt_d, wt_r)
                cdma(shift_sb[:], shift.rearrange("(ob p) -> p ob", p=128))
            elif consts_late and step == 1:
                cdma(wt_d[:, :hk], wt_r[:, :hk])
            elif consts_late and step == 2:
                cdma(wt_d[:, hk:], wt_r[:, hk:])
            elif consts_late and step == 3:
                cdma(shift_sb[:], shift.rearrange("(ob p) -> p ob", p=128))

        emit_consts(0)
        if split_reduce or alt_reduce:
            zeros_col = consts.tile([128, 1], f32)
            nc.gpsimd.memset(zeros_col[:], 0.0)
            scratch = consts.tile([128, SP], f32)

        for b in range(BPC):
            pss = [psump.tile([128, 1], f32, name=f"ps{ob}", tag=f"ps{ob}")
                   for ob in range(NOB)]
            for kt in range(KCH // cpt):
                xt = xin.tile([128, cpt, SP], f32)
                src = x[b, kt * cpt * 128:(kt + 1) * cpt * 128, :] \
                    .rearrange("(c p) s -> p c s", p=128)
                in_eng = nc.scalar if (alt_in_queue and kt % 2) else nc.sync
                if dma_splits == 1:
                    in_eng.dma_start(xt[:], src)
                else:
                    dsp = SP // dma_splits
                    for dd in range(dma_splits):
                        in_eng.dma_start(xt[:, :, dd * dsp:(dd + 1) * dsp],
                                         src[:, :, dd * dsp:(dd + 1) * dsp])
                if b == 0 and kt < 3:
                    emit_consts(kt + 1)
                if alt_reduce and kt % 2 == 1:
                    pta = pooledp.tile([128, 1], f32, name="pta", tag="pta")
                    nc.scalar.activation(scratch[:], xt[:, 0, :], AT.Identity,
                                         bias=zeros_col[:], scale=1.0,
                                         accum_out=pta[:])
                    parts = ((0, pta),)
                elif split_reduce:
                    ptv = pooledp.tile([128, 1], f32, name="ptv", tag="ptv")
                    nc.vector.reduce_sum(ptv[:], xt[:, 0, :],
                                         axis=mybir.AxisListType.X)
                    pta = pooledp.tile([128, 1], f32, name="pta", tag="pta")
                    nc.scalar.activation(scratch[:], xt[:, 1, :], AT.Identity,
                                         bias=zeros_col[:], scale=1.0,
                                         accum_out=pta[:])
                    parts = ((0, ptv), (1, pta))
                else:
                    pt = pooledp.tile([128, cpt], f32, name="pt", tag="pt")
                    nc.vector.reduce_sum(pt[:], xt[:],
                                         axis=mybir.AxisListType.X)
                    parts = tuple((c, pt[:, c:c + 1]) for c in range(cpt))
                for c, pcol in parts:
                    k = kt * cpt + c
                    for ob in range(NOB):
                        nc.tensor.matmul(
                            pss[ob][:],
                            lhsT=wt_sb[:, k * COUT + ob * 128:
                                       k * COUT + ob * 128 + 128],
                            rhs=pcol[:, 0:1],
                            start=(k == 0),
                            stop=(k == KCH - 1),
                        )
            for ob in range(NOB):
                for h in range(bcast_halves):
                    bc = bcastp.tile([128, hsp], f32, name=f"bc{ob}",
                                     tag="bc")
                    src_b = pss[ob][:].broadcast_to([128, hsp])
                    if ob == 1 and bcast_dve_ob1:
                        nc.vector.tensor_scalar(
                            out=bc[:], in0=src_b,
                            scalar1=shift_sb[:, ob:ob + 1], scalar2=0.0,
                            op0=mybir.AluOpType.add, op1=mybir.AluOpType.max)
                    else:
                        nc.scalar.activation(bc[:], src_b, AT.Relu,
                                             bias=shift_sb[:, ob:ob + 1],
                                             scale=1.0)
                    nc.scalar.dma_start(
                        out[b, ob * 128:(ob + 1) * 128,
                            h * hsp:(h + 1) * hsp], bc[:])

    nc.compile()
    return nc


def _prep_inputs(x, W, gamma, beta, running_mean, running_var):
    scale = np.asarray(gamma, np.float32) / np.sqrt(
        np.asarray(running_var, np.float32) + np.float32(BN_EPS))
    wt = np.ascontiguousarray(
        (np.asarray(W, np.float32) * scale[:, None]).T / np.float32(SP))
    shift = (np.asarray(beta, np.float32)
             - np.asarray(running_mean, np.float32) * scale).astype(np.float32)
    xs = np.ascontiguousarray(np.asarray(x, np.float32)).reshape(
        NCORES, BPC, CIN, SP)
    return [{"x": xs[i], "wt": wt, "shift": shift} for i in range(NCORES)]


def kernel(x, W, gamma, beta, running_mean, running_var):
    from concourse import bass_utils

    if "nc" not in _CACHE:
        _CACHE["nc"] = _build_nc()
    nc = _CACHE["nc"]
    in_maps = _prep_inputs(x, W, gamma, beta, running_mean, running_var)
    res = bass_utils.run_bass_kernel_spmd(nc, in_maps,
                                          core_ids=list(range(NCORES)))
    outs = [res.results[i]["out"] for i in range(NCORES)]
    return np.concatenate(outs, axis=0).reshape(B, COUT, H, W_SP)
